# revision 36
# baseline (speedup 1.0000x reference)
"""Trainium2 Bass kernel for nn_Classifier (attribute-sharded MLP heads).

Reference computation (B=64, C=1280, H=W=7, A=40):
    p   = h_swish(mean(x, axis=(2,3)))            # [B, C]
    h   = h_swish(einsum("bc,acd->bad", p, W1) + b1)
    out = sigmoid(einsum("bac,ac->ba", h, W2) + b2)  # [B, A]

Sharding: 8 cores, each owns A/8 = 5 attribute heads (W1/b1/W2/b2 shards);
x is replicated (pre-transposed on host to [C, B*49] so pooling lands in
the matmul-ready [c, b] orientation with zero on-chip transposes).

All large operands are cast to bf16 on host (measured absmax output error
~3e-5 vs fp32 reference; logits are tiny so sigmoid compresses further).
PSUM accumulation stays fp32.
"""

import sys

for _p in ("/opt/trn_rl_repo",):
    if _p not in sys.path:
        sys.path.insert(0, _p)

from contextlib import ExitStack

import numpy as np
import ml_dtypes

import concourse.bass as bass
import concourse.tile as tile
from concourse import bacc, mybir

# Problem constants (hardcoded per contract)
B = 64          # batch
C = 1280        # channels / features
S = 49          # spatial H*W
A = 40          # total attribute heads
NCORES = 8
AH = A // NCORES  # heads per core = 5
P = 128
KC = C // P       # 10 contraction chunks
NS = [(0, 512), (512, 512), (1024, 256)]  # psum n-chunks of C=1280

BF = mybir.dt.bfloat16
F32 = mybir.dt.float32
AF = mybir.ActivationFunctionType
ALU = mybir.AluOpType

_NC_CACHE = {}


def build_nc(reps=1):
    """Build the per-core Bass program (same program on all 8 cores).

    reps>1 unrolls the whole computation back-to-back (same inputs,
    same output) — used only for steady-state throughput benchmarking.
    """
    nc = bacc.Bacc("TRN2", target_bir_lowering=False, name="attr_mlp")

    xT = nc.dram_tensor("xT", [C, B * S], BF, kind="ExternalInput")
    w1 = nc.dram_tensor("w1", [AH, C, C], BF, kind="ExternalInput")
    b1 = nc.dram_tensor("b1", [AH * C], BF, kind="ExternalInput")
    # W2 broadcast with head-major layout: row a*B+b holds W2[a, :]
    w2b = nc.dram_tensor("w2b", [AH * B, C], BF, kind="ExternalInput")
    b2b = nc.dram_tensor("b2b", [AH * B], F32, kind="ExternalInput")
    # output in [head, batch] layout; host transposes back
    out = nc.dram_tensor("out", [AH, B], F32, kind="ExternalOutput")

    with tile.TileContext(nc) as tc, ExitStack() as ctx:
        const = ctx.enter_context(tc.tile_pool(name="const", bufs=1))
        st = ctx.enter_context(tc.tile_pool(name="st", bufs=2))
        xp = ctx.enter_context(tc.tile_pool(name="xp", bufs=6))
        wg = ctx.enter_context(tc.tile_pool(name="wg", bufs=1))
        wp = ctx.enter_context(tc.tile_pool(name="wp", bufs=8))
        sp = ctx.enter_context(tc.tile_pool(name="sp", bufs=3))
        pp = ctx.enter_context(tc.tile_pool(name="pp", bufs=2, space="PSUM"))

        # head groups: pairs share the 128 partitions (64 each) via PE
        # column tiling; the odd head runs solo on 64 partitions, FIRST,
        # so the group finishing last (exposed tail) is a full-width pair.
        groups = [(4,), (0, 1), (2, 3)]

        # --- constants (loaded once) ---
        ones = const.tile([1, B], BF)
        nc.vector.memset(ones, 1.0)
        half = const.tile([P, 1], F32)  # bias=0.5 for the h_swish Relu
        nc.vector.memset(half, 0.5)
        b1_sb = const.tile([1, AH * C], BF)
        nc.sync.dma_start(b1_sb, b1[None, :])
        w2_g, b2_g = [], []
        for g, hs in enumerate(groups):
            pn = B * len(hs)
            r0 = hs[0] * B
            w2t = const.tile([pn, C], BF, tag=f"w2_{g}")
            nc.sync.dma_start(w2t, w2b[r0:r0 + pn, :])
            b2t = const.tile([pn, 1], F32, tag=f"b2_{g}")
            nc.sync.dma_start(b2t, b2b[r0:r0 + pn, None])
            w2_g.append(w2t)
            b2_g.append(b2t)

        for _rep in range(reps):
            # pT[ci, k, b] = h_swish(mean(x))[b, 128*k + ci]  (matmul lhsT)
            pT = st.tile([P, KC, B], BF, tag="pT")

            # --- stage 1: pooling + h_swish -> pT ---
            # pT holds 49*p*clip(p/6+1/2,0,1); the 1/49 is pre-folded into
            # W1 on the host, so GEMM1 still computes p @ W1.
            for k in range(KC):
                xt = xp.tile([P, B * S], BF, tag="xt")
                nc.sync.dma_start(xt, xT[k * P:(k + 1) * P, :])
                sums = sp.tile([P, B], F32, tag="sums")
                nc.vector.reduce_sum(
                    sums, xt.rearrange("p (b s) -> p b s", s=S),
                    axis=mybir.AxisListType.X,
                )
                t1 = sp.tile([P, B], F32, tag="t1")
                nc.scalar.activation(
                    t1, sums, AF.Relu, bias=half, scale=1.0 / (6.0 * 49.0)
                )
                nc.vector.scalar_tensor_tensor(
                    pT[:, k, :], t1, 1.0, sums, ALU.min, ALU.mult
                )

            # --- stage 2: per-head-group GEMM + h_swish + dot(W2) ---
            for g, hs in enumerate(groups):
                pn = B * len(hs)
                nh = len(hs)
                last = g == len(groups) - 1
                # heads of a pair live on disjoint 64-partition halves of
                # the same PSUM banks (PE column tiling). One psum tile
                # per n-chunk so Tile doesn't serialize cross-chunk
                # PSUM readers.
                pss = []
                for ni, (n0, nn) in enumerate(NS):
                    pst = pp.tile([P, nn], F32, tag=f"ps{ni}", name=f"ps{ni}")
                    pss.append(pst)

                def bias_mms():
                    # b1 via K=1 outer product; issued right after the
                    # k=0 matmuls so they don't trail the weight stream
                    for j, a in enumerate(hs):
                        tp = (0, 64 * j) if j else None
                        for ni, (n0, nn) in enumerate(NS):
                            nc.tensor.matmul(
                                pss[ni][64 * j:64 * j + B, :], ones,
                                b1_sb[:, a * C + n0:a * C + n0 + nn],
                                start=False, stop=False, tile_position=tp,
                                skip_group_check=True,
                            )

                if not last:
                    # one whole-group weight DMA (3.3/6.6 MB, max DMA
                    # efficiency; arrival granularity is irrelevant off
                    # the critical tail)
                    w1g = wg.tile([P, nh * KC, C], BF, tag=f"w1g{g}",
                                  name=f"w1g{g}")
                    src = w1[hs[0]:hs[0] + nh].rearrange(
                        "a (ko p) d -> p (a ko) d", p=P)
                    nc.sync.dma_start(w1g, src)
                    for k in range(KC):
                        for j, a in enumerate(hs):
                            tp = (0, 64 * j) if j else None
                            for ni, (n0, nn) in enumerate(NS):
                                nc.tensor.matmul(
                                    pss[ni][64 * j:64 * j + B, :],
                                    pT[:, k, :],
                                    w1g[:, j * KC + k, n0:n0 + nn],
                                    start=(k == 0), stop=(k == KC - 1),
                                    tile_position=tp,
                                    skip_group_check=True,
                                )
                        if k == 0:
                            bias_mms()
                else:
                    # column-major weight stream: n-chunk ni's
                    # accumulation closes at (ni+1)/3 of this group's
                    # stream, so its evacuation overlaps the remaining
                    # stream; only the last (256-wide) chunk is a tail.
                    # DMAs batch KH k-chunks (640/320 KB) to stay off the
                    # HWDGE descriptor-generation floor.
                    KH = 5
                    for ni, (n0, nn) in enumerate(NS):
                        for j, a in enumerate(hs):
                            tp = (0, 64 * j) if j else None
                            for kh in range(KC // KH):
                                w1kt = wp.tile([P, KH, 512], BF, tag="w1kt")
                                src = w1[a, kh * KH * P:(kh + 1) * KH * P,
                                         n0:n0 + nn].rearrange(
                                    "(ko p) d -> p ko d", p=P)
                                nc.sync.dma_start(w1kt[:, :, :nn], src)
                                for ko in range(KH):
                                    k = kh * KH + ko
                                    nc.tensor.matmul(
                                        pss[ni][64 * j:64 * j + B, :],
                                        pT[:, k, :], w1kt[:, ko, :nn],
                                        start=(k == 0), stop=(k == KC - 1),
                                        tile_position=tp,
                                        skip_group_check=True,
                                    )
                                    if k == 0:
                                        nc.tensor.matmul(
                                            pss[ni][64 * j:64 * j + B, :],
                                            ones,
                                            b1_sb[:, a * C + n0:
                                                  a * C + n0 + nn],
                                            start=False, stop=False,
                                            tile_position=tp,
                                            skip_group_check=True,
                                        )
                # evacuation per n-chunk; chains pipeline across chunks:
                #   t1h = Relu(z/6 + 1/2)            [ACT, psum read]
                #   t2w = min(t1h, 1) * w2           [DVE stt]
                #   scr = z * t2w; rpart = sum(scr)  [DVE stt, psum read]
                rpart = st.tile([P, len(NS)], F32, tag="rpart")
                for ni, (n0, nn) in enumerate(NS):
                    zs = pss[ni][:pn]
                    t1h = sp.tile([P, 512], F32, tag="t1h")
                    nc.scalar.activation(
                        t1h[:pn, :nn], zs, AF.Relu,
                        bias=half[:pn], scale=1.0 / 6.0,
                    )
                    t2w = sp.tile([P, 512], BF, tag="t2w")
                    nc.vector.scalar_tensor_tensor(
                        t2w[:pn, :nn], t1h[:pn, :nn], 1.0,
                        w2_g[g][:, n0:n0 + nn], ALU.min, ALU.mult,
                    )
                    scr = sp.tile([P, 512], F32, tag="scr")
                    nc.vector.scalar_tensor_tensor(
                        scr[:pn, :nn], zs, 1.0, t2w[:pn, :nn],
                        ALU.mult, ALU.mult,
                        accum_out=rpart[:pn, ni:ni + 1],
                    )
                rlog = st.tile([P, 1], F32, tag="rlog")
                nc.vector.reduce_sum(
                    rlog[:pn], rpart[:pn, :], axis=mybir.AxisListType.X
                )
                # sigmoid with fused +b2 (per-partition bias)
                osb = st.tile([P, 1], F32, tag="osb")
                nc.scalar.activation(
                    osb[:pn], rlog[:pn], AF.Sigmoid, bias=b2_g[g]
                )
                dst = out[hs[0]:hs[0] + len(hs), :].rearrange(
                    "h b -> (h b)")[:, None]
                if last:
                    # nothing left on the HWDGE rings to block, and HWDGE
                    # latency (~0.6us) beats SWDGE (~2us) on the tail
                    nc.sync.dma_start(dst, osb[:pn])
                else:
                    # SWDGE store: keeps the tiny result write off the
                    # HWDGE rings so it can't head-of-line-block weights
                    nc.gpsimd.dma_start(dst, osb[:pn])

    nc.compile()
    return nc


def get_nc(reps=1):
    if reps not in _NC_CACHE:
        _NC_CACHE[reps] = build_nc(reps)
    return _NC_CACHE[reps]


def make_in_maps(x, W1, b1, W2, b2):
    bf = ml_dtypes.bfloat16
    x = np.asarray(x, dtype=np.float32)
    W1 = np.asarray(W1, dtype=np.float32)
    b1 = np.asarray(b1, dtype=np.float32)
    W2 = np.asarray(W2, dtype=np.float32)
    b2 = np.asarray(b2, dtype=np.float32)

    # [B, C, H, W] -> [C, B*S], replicated to all cores
    xT = np.ascontiguousarray(
        x.reshape(B, C, S).transpose(1, 0, 2)
    ).reshape(C, B * S).astype(bf)

    in_maps = []
    for core in range(NCORES):
        a0 = core * AH
        w2s = W2[a0:a0 + AH]  # [AH, C]
        in_maps.append({
            "xT": xT,
            # 1/49 of the mean pooling is folded into W1 (pT carries 49*p)
            "w1": np.ascontiguousarray(W1[a0:a0 + AH] * (1.0 / 49.0)).astype(bf),
            "b1": np.ascontiguousarray(b1[a0:a0 + AH]).reshape(AH * C).astype(bf),
            # row a*B+b holds W2[a, :]
            "w2b": np.ascontiguousarray(
                np.broadcast_to(w2s[:, None, :], (AH, B, C)).reshape(AH * B, C)
            ).astype(bf),
            "b2b": np.ascontiguousarray(
                np.broadcast_to(b2[a0:a0 + AH, None], (AH, B)).reshape(AH * B)
            ).astype(np.float32),
        })
    return in_maps


def kernel(x, W1, b1, W2, b2, _trace=False, _tmpdir=None):
    from concourse.bass_utils import run_bass_kernel_spmd

    nc = get_nc()
    in_maps = make_in_maps(x, W1, b1, W2, b2)
    res = run_bass_kernel_spmd(
        nc, in_maps, core_ids=list(range(NCORES)),
        trace=_trace, tmpdir=_tmpdir,
    )
    outs = [np.asarray(res.results[c]["out"], dtype=np.float32).T
            for c in range(NCORES)]  # each [B, AH]
    full = np.concatenate(outs, axis=1)  # [B, A]
    if _trace:
        return full, res
    return full


# revision 38
# speedup vs baseline: 1.1045x; 1.1045x over previous
"""Trainium2 Bass kernel for nn_Classifier (attribute-sharded MLP heads).

Reference computation (B=64, C=1280, H=W=7, A=40):
    p   = h_swish(mean(x, axis=(2,3)))            # [B, C]
    h   = h_swish(einsum("bc,acd->bad", p, W1) + b1)
    out = sigmoid(einsum("bac,ac->ba", h, W2) + b2)  # [B, A]

Sharding: 8 cores, each owns A/8 = 5 attribute heads (W1/b1/W2/b2 shards);
x is replicated (pre-transposed on host to [C, B*49] so pooling lands in
the matmul-ready [c, b] orientation with zero on-chip transposes).

All large operands are cast to bf16 on host (measured absmax output error
~3e-5 vs fp32 reference; logits are tiny so sigmoid compresses further).
PSUM accumulation stays fp32.
"""

import sys

for _p in ("/opt/trn_rl_repo",):
    if _p not in sys.path:
        sys.path.insert(0, _p)

from contextlib import ExitStack

import numpy as np
import ml_dtypes

import concourse.bass as bass
import concourse.tile as tile
from concourse import bacc, mybir

# Problem constants (hardcoded per contract)
B = 64          # batch
C = 1280        # channels / features
S = 49          # spatial H*W
A = 40          # total attribute heads
NCORES = 8
AH = A // NCORES  # heads per core = 5
P = 128
KC = C // P       # 10 contraction chunks
NS = [(0, 512), (512, 512), (1024, 256)]  # psum n-chunks of C=1280

BF = mybir.dt.bfloat16
F32 = mybir.dt.float32
AF = mybir.ActivationFunctionType
ALU = mybir.AluOpType

_NC_CACHE = {}


def build_nc(reps=1):
    """Build the per-core Bass program (same program on all 8 cores).

    reps>1 unrolls the whole computation back-to-back (same inputs,
    same output) — used only for steady-state throughput benchmarking.
    """
    nc = bacc.Bacc("TRN2", target_bir_lowering=False, name="attr_mlp")

    xT = nc.dram_tensor("xT", [C, B * S], BF, kind="ExternalInput")
    w1 = nc.dram_tensor("w1", [AH, C, C], BF, kind="ExternalInput")
    b1 = nc.dram_tensor("b1", [AH * C], BF, kind="ExternalInput")
    # W2 broadcast with head-major layout: row a*B+b holds W2[a, :]
    w2b = nc.dram_tensor("w2b", [AH * B, C], BF, kind="ExternalInput")
    b2b = nc.dram_tensor("b2b", [AH * B], F32, kind="ExternalInput")
    # output in [head, batch] layout; host transposes back
    out = nc.dram_tensor("out", [AH, B], F32, kind="ExternalOutput")

    with tile.TileContext(nc) as tc, ExitStack() as ctx:
        const = ctx.enter_context(tc.tile_pool(name="const", bufs=1))
        st = ctx.enter_context(tc.tile_pool(name="st", bufs=2))
        xp = ctx.enter_context(tc.tile_pool(name="xp", bufs=5))
        wg = ctx.enter_context(tc.tile_pool(name="wg", bufs=1))
        wp = ctx.enter_context(tc.tile_pool(name="wp", bufs=10))
        sp = ctx.enter_context(tc.tile_pool(name="sp", bufs=3))
        pp = ctx.enter_context(tc.tile_pool(name="pp", bufs=2, space="PSUM"))

        # head groups: pairs share the 128 partitions (64 each) via PE
        # column tiling; the odd head runs solo on 64 partitions, FIRST,
        # so the group finishing last (exposed tail) is a full-width pair.
        groups = [(4,), (0, 1), (2, 3)]

        # --- constants (loaded once) ---
        ones = const.tile([1, B], BF)
        nc.vector.memset(ones, 1.0)
        half = const.tile([P, 1], F32)  # bias=0.5 for the h_swish Relu
        nc.vector.memset(half, 0.5)
        b1_sb = const.tile([1, AH * C], BF)
        # constants ride the ACT HWDGE ring so they can't head-of-line
        # block the x/W1 stream on the SP ring
        nc.scalar.dma_start(b1_sb, b1[None, :])
        w2_g, b2_g = [], []
        for g, hs in enumerate(groups):
            pn = B * len(hs)
            r0 = hs[0] * B
            w2t = const.tile([pn, C], BF, tag=f"w2_{g}")
            nc.scalar.dma_start(w2t, w2b[r0:r0 + pn, :])
            b2t = const.tile([pn, 1], F32, tag=f"b2_{g}")
            nc.scalar.dma_start(b2t, b2b[r0:r0 + pn, None])
            w2_g.append(w2t)
            b2_g.append(b2t)

        for _rep in range(reps):
            # pT[ci, k, b] = h_swish(mean(x))[b, 128*k + ci]  (matmul lhsT)
            pT = st.tile([P, KC, B], BF, tag="pT")

            # --- stage 1: pooling + h_swish -> pT ---
            # pT holds 49*p*clip(p/6+1/2,0,1); the 1/49 is pre-folded into
            # W1 on the host, so GEMM1 still computes p @ W1.
            for k in range(KC):
                xt = xp.tile([P, B * S], BF, tag="xt")
                nc.sync.dma_start(xt, xT[k * P:(k + 1) * P, :])
                sums = sp.tile([P, B], F32, tag="sums")
                nc.vector.reduce_sum(
                    sums, xt.rearrange("p (b s) -> p b s", s=S),
                    axis=mybir.AxisListType.X,
                )
                t1 = sp.tile([P, B], F32, tag="t1")
                nc.scalar.activation(
                    t1, sums, AF.Relu, bias=half, scale=1.0 / (6.0 * 49.0)
                )
                nc.vector.scalar_tensor_tensor(
                    pT[:, k, :], t1, 1.0, sums, ALU.min, ALU.mult
                )

            # --- stage 2: per-head-group GEMM + h_swish + dot(W2) ---
            for g, hs in enumerate(groups):
                pn = B * len(hs)
                nh = len(hs)
                last = g == len(groups) - 1
                # heads of a pair live on disjoint 64-partition halves of
                # the same PSUM banks (PE column tiling). One psum tile
                # per n-chunk so Tile doesn't serialize cross-chunk
                # PSUM readers.
                pss = []
                for ni, (n0, nn) in enumerate(NS):
                    pst = pp.tile([P, nn], F32, tag=f"ps{ni}", name=f"ps{ni}")
                    pss.append(pst)

                def bias_mms():
                    # b1 via K=1 outer product; issued right after the
                    # k=0 matmuls so they don't trail the weight stream
                    for j, a in enumerate(hs):
                        tp = (0, 64 * j) if j else None
                        for ni, (n0, nn) in enumerate(NS):
                            nc.tensor.matmul(
                                pss[ni][64 * j:64 * j + B, :], ones,
                                b1_sb[:, a * C + n0:a * C + n0 + nn],
                                start=False, stop=False, tile_position=tp,
                                skip_group_check=True,
                            )

                if not last:
                    # one whole-group weight DMA (3.3/6.6 MB, max DMA
                    # efficiency; arrival granularity is irrelevant off
                    # the critical tail)
                    w1g = wg.tile([P, nh * KC, C], BF, tag=f"w1g{g}",
                                  name=f"w1g{g}")
                    src = w1[hs[0]:hs[0] + nh].rearrange(
                        "a (ko p) d -> p (a ko) d", p=P)
                    nc.sync.dma_start(w1g, src)
                    for k in range(KC):
                        for j, a in enumerate(hs):
                            tp = (0, 64 * j) if j else None
                            for ni, (n0, nn) in enumerate(NS):
                                nc.tensor.matmul(
                                    pss[ni][64 * j:64 * j + B, :],
                                    pT[:, k, :],
                                    w1g[:, j * KC + k, n0:n0 + nn],
                                    start=(k == 0), stop=(k == KC - 1),
                                    tile_position=tp,
                                    skip_group_check=True,
                                )
                        if k == 0:
                            bias_mms()
                else:
                    # column-major weight stream: n-chunk ni's
                    # accumulation closes at (ni+1)/3 of this group's
                    # stream, so its evacuation overlaps the remaining
                    # stream; only the last (256-wide) chunk is a tail.
                    # DMAs batch KH k-chunks (640/320 KB) to stay off the
                    # HWDGE descriptor-generation floor.
                    KH = 5
                    for ni, (n0, nn) in enumerate(NS):
                        for j, a in enumerate(hs):
                            tp = (0, 64 * j) if j else None
                            for kh in range(KC // KH):
                                w1kt = wp.tile([P, KH, 512], BF, tag="w1kt")
                                src = w1[a, kh * KH * P:(kh + 1) * KH * P,
                                         n0:n0 + nn].rearrange(
                                    "(ko p) d -> p ko d", p=P)
                                nc.sync.dma_start(w1kt[:, :, :nn], src)
                                for ko in range(KH):
                                    k = kh * KH + ko
                                    nc.tensor.matmul(
                                        pss[ni][64 * j:64 * j + B, :],
                                        pT[:, k, :], w1kt[:, ko, :nn],
                                        start=(k == 0), stop=(k == KC - 1),
                                        tile_position=tp,
                                        skip_group_check=True,
                                    )
                                    if k == 0:
                                        nc.tensor.matmul(
                                            pss[ni][64 * j:64 * j + B, :],
                                            ones,
                                            b1_sb[:, a * C + n0:
                                                  a * C + n0 + nn],
                                            start=False, stop=False,
                                            tile_position=tp,
                                            skip_group_check=True,
                                        )
                # evacuation per n-chunk; chains pipeline across chunks:
                #   t1h = Relu(z/6 + 1/2)            [ACT, psum read]
                #   t2w = min(t1h, 1) * w2           [DVE stt]
                #   scr = z * t2w; rpart = sum(scr)  [DVE stt, psum read]
                rpart = st.tile([P, len(NS)], F32, tag="rpart")
                for ni, (n0, nn) in enumerate(NS):
                    zs = pss[ni][:pn]
                    t1h = sp.tile([P, 512], F32, tag="t1h")
                    nc.scalar.activation(
                        t1h[:pn, :nn], zs, AF.Relu,
                        bias=half[:pn], scale=1.0 / 6.0,
                    )
                    t2w = sp.tile([P, 512], BF, tag="t2w")
                    nc.vector.scalar_tensor_tensor(
                        t2w[:pn, :nn], t1h[:pn, :nn], 1.0,
                        w2_g[g][:, n0:n0 + nn], ALU.min, ALU.mult,
                    )
                    scr = sp.tile([P, 512], F32, tag="scr")
                    nc.vector.scalar_tensor_tensor(
                        scr[:pn, :nn], zs, 1.0, t2w[:pn, :nn],
                        ALU.mult, ALU.mult,
                        accum_out=rpart[:pn, ni:ni + 1],
                    )
                rlog = st.tile([P, 1], F32, tag="rlog")
                nc.vector.reduce_sum(
                    rlog[:pn], rpart[:pn, :], axis=mybir.AxisListType.X
                )
                # sigmoid with fused +b2 (per-partition bias)
                osb = st.tile([P, 1], F32, tag="osb")
                nc.scalar.activation(
                    osb[:pn], rlog[:pn], AF.Sigmoid, bias=b2_g[g]
                )
                dst = out[hs[0]:hs[0] + len(hs), :].rearrange(
                    "h b -> (h b)")[:, None]
                if last:
                    # nothing left on the HWDGE rings to block, and HWDGE
                    # latency (~0.6us) beats SWDGE (~2us) on the tail
                    nc.sync.dma_start(dst, osb[:pn])
                else:
                    # SWDGE store: keeps the tiny result write off the
                    # HWDGE rings so it can't head-of-line-block weights
                    nc.gpsimd.dma_start(dst, osb[:pn])

    nc.compile()
    return nc


def get_nc(reps=1):
    if reps not in _NC_CACHE:
        _NC_CACHE[reps] = build_nc(reps)
    return _NC_CACHE[reps]


def make_in_maps(x, W1, b1, W2, b2):
    bf = ml_dtypes.bfloat16
    x = np.asarray(x, dtype=np.float32)
    W1 = np.asarray(W1, dtype=np.float32)
    b1 = np.asarray(b1, dtype=np.float32)
    W2 = np.asarray(W2, dtype=np.float32)
    b2 = np.asarray(b2, dtype=np.float32)

    # [B, C, H, W] -> [C, B*S], replicated to all cores
    xT = np.ascontiguousarray(
        x.reshape(B, C, S).transpose(1, 0, 2)
    ).reshape(C, B * S).astype(bf)

    in_maps = []
    for core in range(NCORES):
        a0 = core * AH
        w2s = W2[a0:a0 + AH]  # [AH, C]
        in_maps.append({
            "xT": xT,
            # 1/49 of the mean pooling is folded into W1 (pT carries 49*p)
            "w1": np.ascontiguousarray(W1[a0:a0 + AH] * (1.0 / 49.0)).astype(bf),
            "b1": np.ascontiguousarray(b1[a0:a0 + AH]).reshape(AH * C).astype(bf),
            # row a*B+b holds W2[a, :]
            "w2b": np.ascontiguousarray(
                np.broadcast_to(w2s[:, None, :], (AH, B, C)).reshape(AH * B, C)
            ).astype(bf),
            "b2b": np.ascontiguousarray(
                np.broadcast_to(b2[a0:a0 + AH, None], (AH, B)).reshape(AH * B)
            ).astype(np.float32),
        })
    return in_maps


def kernel(x, W1, b1, W2, b2, _trace=False, _tmpdir=None):
    from concourse.bass_utils import run_bass_kernel_spmd

    nc = get_nc()
    in_maps = make_in_maps(x, W1, b1, W2, b2)
    res = run_bass_kernel_spmd(
        nc, in_maps, core_ids=list(range(NCORES)),
        trace=_trace, tmpdir=_tmpdir,
    )
    outs = [np.asarray(res.results[c]["out"], dtype=np.float32).T
            for c in range(NCORES)]  # each [B, AH]
    full = np.concatenate(outs, axis=1)  # [B, A]
    if _trace:
        return full, res
    return full
